# revision 1
# baseline (speedup 1.0000x reference)
"""Trainium2 Bass kernel: causal GQA self-attention (B=1, S=2048, D=2048,
H=16 q-heads, 4 kv-heads, head_dim=128) with q/k RMS-norm, full-head RoPE,
per-head q gain, and output projection.

Sharding: tensor-parallel over 8 NeuronCores. Core i owns q-heads {2i, 2i+1}
and kv-head i//2: it computes its two heads' attention output and a partial
output projection (columns 256i..256i+256 of the y-concat contracted against
Wproj), returning a full-shape [2048, 2048] f32 partial that the host sums
across cores (the "all-reduce").

Everything runs in bf16 on the PE array with f32 PSUM accumulation; the
normalizations are applied in f32 (k's 1/rms rides the softmax-exp's
per-partition scale, q's is multiplied in f32 before rounding to bf16).
"""

import math
from contextlib import ExitStack

import numpy as np
import ml_dtypes

import concourse.bass as bass
import concourse.tile as tile
from concourse import mybir, library_config
from concourse.bass_isa import ReduceOp
from concourse.bass_utils import run_bass_kernel_spmd

BF16 = mybir.dt.bfloat16
F32 = mybir.dt.float32
NP_BF16 = ml_dtypes.bfloat16

S = 2048
D = 2048
H = 16
HKV = 4
HD = 128
NCORES = 8
QH_PER_CORE = H // NCORES          # 2
M_PER_CORE = QH_PER_CORE * HD      # 256
NT = S // 128                      # 16 s-tiles / d-blocks
BASE = 10000.0
EPS = float(np.finfo(np.float32).eps)

AF = mybir.ActivationFunctionType
ALU = mybir.AluOpType


_MAXW = 1  # max sync-wait commands this walrus accepts per instruction


def _install_drain_split_patch():
    """The walrus build here only accepts <=2 sync-wait commands per
    instruction.  Tile attaches one wait per producer semaphore, which can be
    many.  Split the excess onto same-engine NoOps committed immediately
    before the instruction (same program point, so ordering semantics are
    unchanged), and likewise chain the kernel-tail drain."""
    from concourse.vector_clock import ScopedClock
    import bass_rust

    if getattr(tile.TileContext, "_drain_split_patched", False):
        return

    orig_commit = tile.TileContext._commit_instruction

    def _commit_instruction(self, inst, lazy_reg_writes=True):
        si = getattr(inst, "sync_info", None)
        if (si is not None and si.on_wait and len(si.on_wait) > _MAXW
                and inst.engine != mybir.EngineType.Unassigned
                and isinstance(inst, mybir.Instruction)):
            waits = list(si.on_wait)
            excess, keep = waits[:-_MAXW], waits[-_MAXW:]
            for k in range(0, len(excess), _MAXW):
                nop = mybir.InstNoOp(
                    name=f"{inst.name}-wsplit{k}", ins=[], outs=[])
                nop.engine = inst.engine
                nop.sync_info = bass_rust.SyncInfo(
                    on_wait=excess[k:k + _MAXW], on_update=[])
                self._add_instruction(nop)
            si.on_wait = keep
            inst.sync_info = si
        return orig_commit(self, inst, lazy_reg_writes)

    tile.TileContext._commit_instruction = _commit_instruction

    def _drain_and_barrier(self, tick_clock, wait_clock):
        nc = self.nc
        drain_inst = nc.sync.drain()
        wait_clock.add_sem_waits(
            drain_inst.ins, ScopedClock({None: tick_clock.global_clock}))
        mi = drain_inst.ins
        si = mi.sync_info
        if si is not None and si.on_wait and len(si.on_wait) > 1:
            waits = list(si.on_wait)
            si.on_wait = waits[:1]
            mi.sync_info = si
            for w in waits[1:]:
                d2 = nc.sync.drain()
                s2 = d2.ins.sync_info
                if s2 is None:
                    s2 = type(si)(on_wait=[w], on_update=[])
                else:
                    s2.on_wait = [w]
                d2.ins.sync_info = s2
        nc.all_engine_barrier()
        assert self.sems is not None
        popped = nc._tile_sem_poison_stack.pop()
        assert popped is self._sem_poison
        nc.clear_and_free_semaphores(list(self.sems.allocated().values()))
        nc.all_engine_barrier()

    tile.TileContext._drain_and_barrier = _drain_and_barrier
    tile.TileContext._drain_split_patched = True


_install_drain_split_patch()


def _build_program(loop_reps=None):
    nc = bass.Bass()

    # ---- DRAM I/O ----
    xt_d = nc.declare_dram_parameter("xt", [D, S], BF16, isOutput=False)
    wqt_d = nc.declare_dram_parameter("wqt", [D, M_PER_CORE], BF16, isOutput=False)
    wkt_d = nc.declare_dram_parameter("wkt", [D, HD], BF16, isOutput=False)
    wvt_d = nc.declare_dram_parameter("wvt", [D, HD], BF16, isOutput=False)
    pt_d = nc.declare_dram_parameter("pt", [M_PER_CORE, D], BF16, isOutput=False)
    cost_d = nc.declare_dram_parameter("cost", [HD, S], BF16, isOutput=False)
    sint_d = nc.declare_dram_parameter("sint", [HD, S], BF16, isOutput=False)
    mask_d = nc.declare_dram_parameter("maskd", [128, 128], BF16, isOutput=False)
    ident_d = nc.declare_dram_parameter("ident", [128, 128], BF16, isOutput=False)
    qkc_d = nc.declare_dram_parameter("qkc", [128, 6], F32, isOutput=False)
    out_d = nc.declare_dram_parameter("partial", [S, D], F32, isOutput=True)

    with tile.TileContext(nc) as tc:
        with ExitStack() as ctx:
            pers = ctx.enter_context(tc.tile_pool(name="pers", bufs=1))
            tp_f32 = ctx.enter_context(tc.tile_pool(name="tp_f32", bufs=2))
            tp_half = ctx.enter_context(tc.tile_pool(name="tp_half", bufs=1))
            tp_q = ctx.enter_context(tc.tile_pool(name="tp_q", bufs=2))
            tp_probs = ctx.enter_context(tc.tile_pool(name="tp_probs", bufs=6))
            tp_row = ctx.enter_context(tc.tile_pool(name="tp_row", bufs=1))
            tp_y = ctx.enter_context(tc.tile_pool(name="tp_y", bufs=3))
            tp_rec = ctx.enter_context(tc.tile_pool(name="tp_rec", bufs=3))
            tp_out = ctx.enter_context(tc.tile_pool(name="tp_out", bufs=2))
            ps_big = ctx.enter_context(
                tc.tile_pool(name="ps_big", bufs=3, space="PSUM"))
            ps_proj = ctx.enter_context(
                tc.tile_pool(name="ps_proj", bufs=2, space="PSUM"))
            ps_sm = ctx.enter_context(
                tc.tile_pool(name="ps_sm", bufs=3, space="PSUM"))
            dram = ctx.enter_context(
                tc.tile_pool(name="dram", bufs=1, space="DRAM"))

            # ---- Phase A: load everything ----
            # xt (critical path for the first projections) on the SP HWDGE
            # ring interleaved with wqt on the ACT ring; everything else on
            # the gpsimd SWDGE queues so the rings don't serialize behind
            # each other.
            # consolidated weight loads first on the ACT ring (q-proj needs
            # wqt immediately), then xt tiles alternating between the SP and
            # ACT rings so the PE never catches up with the loader.
            wqt_all = pers.tile([128, NT * M_PER_CORE], BF16, tag="wqt_all",
                                name="wqt_all")
            nc.scalar.dma_start(
                wqt_all[:].rearrange("p (t c) -> p t c", c=M_PER_CORE),
                wqt_d[:].rearrange("(t p) c -> p t c", p=128))
            xt = []
            for t in range(NT):
                xti = pers.tile([128, S], BF16, tag=f"xt{t}", name=f"xt{t}")
                eng = nc.sync if t % 2 == 0 else nc.scalar
                eng.dma_start(xti[:], xt_d[128 * t:128 * (t + 1), :])
                xt.append(xti)
            wkt_all = pers.tile([128, NT * HD], BF16, tag="wkt_all",
                                name="wkt_all")
            nc.gpsimd.dma_start(
                wkt_all[:].rearrange("p (t c) -> p t c", c=HD),
                wkt_d[:].rearrange("(t p) c -> p t c", p=128))
            wvt_all = pers.tile([128, NT * HD], BF16, tag="wvt_all",
                                name="wvt_all")
            nc.gpsimd.dma_start(
                wvt_all[:].rearrange("p (t c) -> p t c", c=HD),
                wvt_d[:].rearrange("(t p) c -> p t c", p=128))
            pt = []
            for h in range(QH_PER_CORE):
                w = pers.tile([128, D], BF16, tag=f"pt{h}", name=f"pt{h}")
                nc.gpsimd.dma_start(w[:], pt_d[128 * h:128 * (h + 1), :])
                pt.append(w)
            cost = pers.tile([HD, S], BF16, tag="cost", name="cost")
            nc.gpsimd.dma_start(cost[:], cost_d[:])
            sint = pers.tile([HD, S], BF16, tag="sint", name="sint")
            nc.gpsimd.dma_start(sint[:], sint_d[:])
            maskd = pers.tile([128, 128], BF16, tag="maskd", name="maskd")
            nc.gpsimd.dma_start(maskd[:], mask_d[:])
            ident = pers.tile([128, 128], BF16, tag="ident", name="ident")
            nc.gpsimd.dma_start(ident[:], ident_d[:])
            qkc = pers.tile([128, 6], F32, tag="qkc", name="qkc")
            nc.gpsimd.dma_start(qkc[:], qkc_d[:])
            ones_col = pers.tile([128, 1], BF16, tag="ones_col",
                                 name="ones_col")
            nc.vector.memset(ones_col[:], 1.0)

            # persistent results of phase B
            qTn = [pers.tile([128, S], BF16, tag=f"qTn{h}", name=f"qTn{h}")
                   for h in range(QH_PER_CORE)]
            kTr = pers.tile([128, S], BF16, tag="kTr", name="kTr")
            invk_cols = pers.tile([128, NT], F32, tag="invk_cols", name="invk_cols")
            vplus = [pers.tile([128, 129], BF16, tag=f"vplus{t}", name=f"vplus{t}")
                     for t in range(NT)]
            yT = [pers.tile([128, S], BF16, tag=f"yT{h}", name=f"yT{h}")
                  for h in range(QH_PER_CORE)]

            def proj_unit(w_ap_of):
                """x @ W.T in transposed layout: returns list of psum chunk
                tiles [128, 512] covering out[hd, s].  w_ap_of(dblk) gives the
                [128, 128] lhsT slice for d-block dblk.

                Two passes of two chunks, d-block outermost within a pass:
                PE can start as soon as xt[0] lands, each LDWEIGHTS feeds two
                matmuls, and only two PSUM slots are held at a time."""
                t = tp_q.tile([128, S], BF16, tag="traw", name="traw")
                for half in range(2):
                    pair = [ps_proj.tile([128, 512], F32, tag="pj", name="pj")
                            for _ in range(2)]
                    for dblk in range(NT):
                        for c in range(2):
                            cc = 2 * half + c
                            nc.tensor.matmul(
                                pair[c][:],
                                w_ap_of(dblk),
                                xt[dblk][:, 512 * cc:512 * (cc + 1)],
                                start=(dblk == 0),
                                stop=(dblk == NT - 1),
                            )
                    for c in range(2):
                        cc = 2 * half + c
                        nc.vector.tensor_copy(
                            t[:, 512 * cc:512 * (cc + 1)], pair[c][:])
                return t

            def rope(src, dst):
                """dst = rotate(src): dst = src*cosF + halfswap(src)*sinF.

                cosF = [cos; cos], sinF = [sin; -sin]; the half-swap (the only
                cross-partition move) goes through SBUF->SBUF DMA since DVE
                lanes are partition-locked."""
                sw = tp_q.tile([128, S], BF16, tag="qsw", name="qsw")
                nc.sync.dma_start(sw[0:64, :], src[64:128, :])
                nc.sync.dma_start(sw[64:128, :], src[0:64, :])
                t1 = tp_half.tile([128, S], BF16, tag="rt1", name="rt1")
                t2 = tp_half.tile([128, S], BF16, tag="rt2", name="rt2")
                nc.vector.tensor_mul(t1[:], src[:], cost[:])
                nc.vector.tensor_mul(t2[:], sw[:], sint[:])
                nc.vector.tensor_add(dst[:], t1[:], t2[:])

            def inv_rms_row(src, scale_ap, bias_ap, rowname):
                """1/sqrt(colsum(src^2)*scale + bias) as a DRAM row [1, S].

                Column sums over the 128 partitions go through the PE (ones
                lhsT); sqrt/recip run on the [1, S] row."""
                sq = tp_q.tile([128, S], BF16, tag="qsw", name="sq")
                nc.vector.tensor_mul(sq[:], src[:], src[:])
                rrow = tp_row.tile([1, S], F32, tag="rrow", name="rrow")
                for c in range(4):
                    rp = ps_sm.tile([128, 512], F32, tag="sm", name="rowps")
                    nc.tensor.matmul(rp[0:1, :], ones_col[:],
                                     sq[:, 512 * c:512 * (c + 1)],
                                     start=True, stop=True)
                    nc.scalar.activation(rrow[0:1, 512 * c:512 * (c + 1)],
                                         rp[0:1, :], AF.Sqrt,
                                         scale=scale_ap, bias=bias_ap)
                nc.vector.reciprocal(rrow[:], rrow[:])
                rowdr = dram.tile([1, S], F32, tag="rowdr", name=rowname,
                                  bufs=2)
                nc.scalar.dma_start(rowdr[:], rrow[:])
                return rowdr

            def q_unit(h):
                qraw = proj_unit(
                    lambda d: wqt_all[:, M_PER_CORE * d + 128 * h:
                                      M_PER_CORE * d + 128 * (h + 1)])
                qTr = tp_q.tile([128, S], BF16, tag="tr2", name="tr2")
                rope(qraw, qTr)
                rowdr = inv_rms_row(qTr, qkc[0:1, 2 * h:2 * h + 1],
                                    qkc[0:1, 2 * h + 1:2 * h + 2],
                                    f"invq_row{h}")
                inv_bc = tp_f32.tile([128, S], F32, tag="nchain",
                                     name="inv_bc")
                nc.scalar.dma_start(inv_bc[:],
                                    rowdr[0:1, :].broadcast_to([128, S]))
                nc.vector.tensor_mul(qTn[h][:], qTr[:], inv_bc[:])

            def _phases_bc():
                # ---- Phase B: projections + norms + rope ----
                # order q0, k, v, q1: head-0 attention only needs qTn[0], k
                # and v, so it can start while q1's norm chain drains.
                q_unit(0)

                # k unit
                kraw = proj_unit(
                    lambda d: wkt_all[:, HD * d:HD * (d + 1)])
                rope(kraw, kTr)
                rowdr = inv_rms_row(kTr, qkc[0:1, 4:5], qkc[0:1, 5:6],
                                    "invk_row")
                nc.sync.dma_start(
                    invk_cols[:],
                    rowdr[0:1, :].rearrange("one (j p) -> p (one j)", p=128))

                q_unit(1)

                # preload the ACT exp table set (Sqrt->Exp switch costs
                # ~2.7us) while PE is still busy with projections
                warm = tp_rec.tile([128, 1], F32, tag="rec", name="warm")
                nc.scalar.activation(warm[:], qkc[:, 0:1], AF.Exp)

                # v unit
                vT = proj_unit(
                    lambda d: wvt_all[:, HD * d:HD * (d + 1)])
                for t in range(NT):
                    trp = ps_sm.tile([128, 128], BF16, tag="sm", name="sm")
                    nc.tensor.transpose(trp[:], vT[:, 128 * t:128 * (t + 1)], ident[:])
                    nc.vector.tensor_copy(vplus[t][:, 0:128], trp[:])
                    nc.vector.memset(vplus[t][:, 128:129], 1.0)

                # ---- Phase C: attention, interleaved head/batch order ----
                out_q = [nc.gpsimd, nc.sync, nc.scalar]

                def proj_out(i):
                    """output projection for s-tile i (needs yT of both
                    heads); stores round-robin over the three DMA queues."""
                    for c in range(4):
                        pp = ps_proj.tile([128, 512], F32, tag="pj", name="pj")
                        nc.tensor.matmul(pp[:], yT[0][:, 128 * i:128 * (i + 1)],
                                         pt[0][:, 512 * c:512 * (c + 1)],
                                         start=True, stop=False)
                        nc.tensor.matmul(pp[:], yT[1][:, 128 * i:128 * (i + 1)],
                                         pt[1][:, 512 * c:512 * (c + 1)],
                                         start=False, stop=True)
                        ob = tp_out.tile([128, 512], F32, tag="ob", name="ob")
                        if c % 2 == 0:
                            nc.vector.tensor_copy(ob[:], pp[:])
                        else:
                            nc.scalar.copy(ob[:], pp[:])
                        out_q[(4 * i + c) % 3].dma_start(
                            out_d[128 * i:128 * (i + 1), 512 * c:512 * (c + 1)],
                            ob[:])

                # (h0, A), (h1, A), (h0, B), (h1, B): the output projection
                # for s-tiles 0..7 runs during (h0, B), filling PE's
                # exp-wait gaps instead of piling up in the tail.
                for (h, lo, hi) in ((0, 0, 8), (1, 0, 8), (0, 8, 16),
                                    (1, 8, 16)):
                    nb = (hi - lo + 2) // 3
                    banks = [ps_big.tile([128, 512], F32, tag="big", name="big")
                             for _ in range(nb)]
                    imax_of_bank = [min(lo + 3 * b + 2, hi - 1)
                                    for b in range(nb)]
                    for j in range(hi):
                        qlo = max(lo * 128, j * 128)
                        w = hi * 128 - qlo
                        # per-512-chunk scores + probs tiles: PV on chunk c
                        # doesn't wait for exp of chunk c+1
                        pchunks = []
                        for c in range((w + 511) // 512):
                            cw = min(512, w - 512 * c)
                            sc = ps_sm.tile([128, 512], F32, tag="sm", name="sm")
                            nc.tensor.matmul(
                                sc[:, :cw],
                                kTr[:, 128 * j:128 * (j + 1)],
                                qTn[h][:, qlo + 512 * c: qlo + 512 * c + cw],
                                start=True, stop=True)
                            pc = tp_probs.tile([128, 512], BF16,
                                               tag="probs", name="probs")
                            nc.scalar.activation(
                                pc[:, :cw], sc[:, :cw],
                                AF.Exp, scale=invk_cols[:, j:j + 1])
                            pchunks.append(pc)
                        if j >= lo:
                            nc.vector.tensor_mul(
                                pchunks[0][:, 0:128], pchunks[0][:, 0:128],
                                maskd[:])
                        for i in range(max(lo, j), hi):
                            b, sub = divmod(i - lo, 3)
                            off = 128 * i - qlo
                            nc.tensor.matmul(
                                banks[b][:, 129 * sub:129 * sub + 129],
                                pchunks[off // 512][:, off % 512:off % 512 + 128],
                                vplus[j][:],
                                start=(j == 0 and sub == 0),
                                stop=(j == i and i == imax_of_bank[b]))
                        # extract completed bank (all its accums final)
                        for b in range(nb):
                            if j == imax_of_bank[b] and j >= lo:
                                for i in range(lo + 3 * b,
                                               min(lo + 3 * b + 3, hi)):
                                    sub = (i - lo) % 3
                                    rec = tp_rec.tile([128, 1], F32, tag="rec", name="rec")
                                    nc.vector.reciprocal(
                                        rec[:],
                                        banks[b][:, 129 * sub + 128:
                                                 129 * sub + 129])
                                    y = tp_y.tile([128, 128], BF16, tag="y", name="y")
                                    nc.vector.tensor_scalar_mul(
                                        y[:],
                                        banks[b][:, 129 * sub:129 * sub + 128],
                                        rec[:])
                                    trp = ps_sm.tile([128, 128], BF16, tag="sm", name="sm")
                                    nc.tensor.transpose(trp[:], y[:], ident[:])
                                    nc.vector.tensor_copy(
                                        yT[h][:, 128 * i:128 * (i + 1)], trp[:])
                                    if h == 1:
                                        proj_out(i)

            if loop_reps is None:
                _phases_bc()
            else:
                with tc.For_i(0, loop_reps, 1):
                    _phases_bc()
    return nc


_NC_CACHE = None


def _get_nc():
    global _NC_CACHE
    if _NC_CACHE is None:
        _NC_CACHE = _build_program()
    return _NC_CACHE


def _host_prep(x, Wq, Wk, Wv, Wproj, q_gain):
    """Build the 8 per-core input maps (numpy, host side)."""
    x2 = np.asarray(x, np.float32).reshape(S, D)
    xt = np.ascontiguousarray(x2.T).astype(NP_BF16)

    inv_freq = 1.0 / (BASE ** (np.arange(0, HD, 2, dtype=np.float32) / HD))
    t = np.arange(S, dtype=np.float32)
    freqs = np.outer(t, inv_freq)                     # [S, 64]
    cos_h = np.cos(freqs).T                           # [64, S]
    sin_h = np.sin(freqs).T
    cosT = np.ascontiguousarray(
        np.concatenate([cos_h, cos_h], axis=0)).astype(NP_BF16)
    sinT = np.ascontiguousarray(
        np.concatenate([sin_h, -sin_h], axis=0)).astype(NP_BF16)

    maskd = np.triu(np.ones((128, 128), np.float32)).astype(NP_BF16)
    ident = np.eye(128, dtype=np.float32).astype(NP_BF16)

    Wq = np.asarray(Wq, np.float32)
    Wk = np.asarray(Wk, np.float32)
    Wv = np.asarray(Wv, np.float32)
    Wproj = np.asarray(Wproj, np.float32)
    q_gain = np.asarray(q_gain, np.float32)

    in_maps = []
    for core in range(NCORES):
        kv = core // 2
        wqt = np.ascontiguousarray(
            Wq[M_PER_CORE * core:M_PER_CORE * (core + 1), :].T).astype(NP_BF16)
        wkt = np.ascontiguousarray(
            Wk[HD * kv:HD * (kv + 1), :].T).astype(NP_BF16)
        wvt = np.ascontiguousarray(
            Wv[HD * kv:HD * (kv + 1), :].T).astype(NP_BF16)
        ptc = np.ascontiguousarray(
            Wproj[:, M_PER_CORE * core:M_PER_CORE * (core + 1)].T
        ).astype(NP_BF16)
        qkc = np.zeros((128, 6), np.float32)
        qkc[:, 4] = 1.0 / HD
        qkc[:, 5] = EPS
        for h in range(QH_PER_CORE):
            gain = float(q_gain[QH_PER_CORE * core + h])
            c = gain / math.sqrt(HD)
            if abs(c) < 1e-8:
                c = 1e-8
            qkc[:, 2 * h] = 1.0 / (HD * c * c)
            qkc[:, 2 * h + 1] = EPS / (c * c)
        in_maps.append({
            "xt": xt,
            "wqt": wqt,
            "wkt": wkt,
            "wvt": wvt,
            "pt": ptc,
            "cost": cosT,
            "sint": sinT,
            "maskd": maskd,
            "ident": ident,
            "qkc": qkc,
        })
    return in_maps


def kernel(**inputs):
    x = inputs["x"]
    in_maps = _host_prep(x, inputs["Wq"], inputs["Wk"], inputs["Wv"],
                         inputs["Wproj"], inputs["q_gain"])
    nc = _get_nc()
    res = run_bass_kernel_spmd(nc, in_maps, list(range(NCORES)))
    parts = [np.asarray(res.results[i]["partial"]) for i in range(NCORES)]
    out = np.sum(np.stack(parts, 0), axis=0, dtype=np.float64)
    return out.astype(np.float32).reshape(1, S, D)



# revision 69
# speedup vs baseline: 1.6437x; 1.6437x over previous
"""Trainium2 Bass kernel: causal GQA self-attention (B=1, S=2048, D=2048,
H=16 q-heads, 4 kv-heads, head_dim=128) with q/k RMS-norm, full-head RoPE,
per-head q gain, and output projection.

Sharding: tensor-parallel over 8 NeuronCores. Core i owns q-heads {2i, 2i+1}
and kv-head i//2: it computes its two heads' attention output and a partial
output projection (columns 256i..256i+256 of the y-concat contracted against
Wproj), returning a full-shape [2048, 2048] bf16 partial that the host sums
across cores (the "all-reduce").

Schedule, built around keeping the PE (the bottleneck engine) gap-free:
  A: loads striped over the three HWDGE rings, first-needed first (q0/k
     weights + x tile 0 land ~3.5us in); phase-C-only tensors queue behind
     the x tiles on the same rings.
  B: projection pass 1 (q-head0 + k over all 8 PSUM banks) paced by the x
     loader, then sub-pass 2a (v) and 2b (q-head1) on 4 banks each. The
     norm/rope chains run chunk-wise behind the PE; their column-sum and
     row-broadcast matmuls (a rank-1 ones @ row product, replacing a DRAM
     round-trip) fill d-block boundaries of 2a/2b, as do head-0's scores +
     exps for query blocks 0/1 (their probs park in SBUF) and the v
     transposes.
  C: attention in four query blocks of 4 tiles per head. Pair 0 (h0 blocks
     0/1) is pure PV against the parked probs; pairs (h1 b0/b1), (h0+h1 b2),
     (h0+h1 b3) run j-interleaved with scores issued one step ahead of PV so
     each stream's exp hides under the other's PE work. Output-projection
     chunks spread one per stream step from a pending queue (tiles enqueue
     as both heads' y land), staging to bf16 and DMAing a full row-tile out.
"""

import math
from contextlib import ExitStack

import numpy as np
import ml_dtypes

import concourse.bass as bass
import concourse.tile as tile
from concourse import mybir, library_config
from concourse.bass_isa import ReduceOp
from concourse.bass_utils import run_bass_kernel_spmd

BF16 = mybir.dt.bfloat16
F32 = mybir.dt.float32
NP_BF16 = ml_dtypes.bfloat16

S = 2048
D = 2048
H = 16
HKV = 4
HD = 128
NCORES = 8
QH_PER_CORE = H // NCORES          # 2
M_PER_CORE = QH_PER_CORE * HD      # 256
NT = S // 128                      # 16 s-tiles / d-blocks
BASE = 10000.0
EPS = float(np.finfo(np.float32).eps)

AF = mybir.ActivationFunctionType
ALU = mybir.AluOpType


_MAXW = 1  # max sync-wait commands this walrus accepts per instruction


def _install_drain_split_patch():
    """The walrus build here only accepts <=2 sync-wait commands per
    instruction.  Tile attaches one wait per producer semaphore, which can be
    many.  Split the excess onto same-engine NoOps committed immediately
    before the instruction (same program point, so ordering semantics are
    unchanged), and likewise chain the kernel-tail drain."""
    from concourse.vector_clock import ScopedClock
    import bass_rust

    if getattr(tile.TileContext, "_drain_split_patched", False):
        return

    orig_commit = tile.TileContext._commit_instruction

    def _commit_instruction(self, inst, lazy_reg_writes=True):
        si = getattr(inst, "sync_info", None)
        if (si is not None and si.on_wait and len(si.on_wait) > _MAXW
                and inst.engine != mybir.EngineType.Unassigned
                and isinstance(inst, mybir.Instruction)):
            waits = list(si.on_wait)
            excess, keep = waits[:-_MAXW], waits[-_MAXW:]
            for k in range(0, len(excess), _MAXW):
                nop = mybir.InstNoOp(
                    name=f"{inst.name}-wsplit{k}", ins=[], outs=[])
                nop.engine = inst.engine
                nop.sync_info = bass_rust.SyncInfo(
                    on_wait=excess[k:k + _MAXW], on_update=[])
                self._add_instruction(nop)
            si.on_wait = keep
            inst.sync_info = si
        return orig_commit(self, inst, lazy_reg_writes)

    tile.TileContext._commit_instruction = _commit_instruction

    def _drain_and_barrier(self, tick_clock, wait_clock):
        nc = self.nc
        drain_inst = nc.sync.drain()
        wait_clock.add_sem_waits(
            drain_inst.ins, ScopedClock({None: tick_clock.global_clock}))
        mi = drain_inst.ins
        si = mi.sync_info
        if si is not None and si.on_wait and len(si.on_wait) > 1:
            waits = list(si.on_wait)
            si.on_wait = waits[:1]
            mi.sync_info = si
            for w in waits[1:]:
                d2 = nc.sync.drain()
                s2 = d2.ins.sync_info
                if s2 is None:
                    s2 = type(si)(on_wait=[w], on_update=[])
                else:
                    s2.on_wait = [w]
                d2.ins.sync_info = s2
        nc.all_engine_barrier()
        assert self.sems is not None
        popped = nc._tile_sem_poison_stack.pop()
        assert popped is self._sem_poison
        nc.clear_and_free_semaphores(list(self.sems.allocated().values()))
        nc.all_engine_barrier()

    tile.TileContext._drain_and_barrier = _drain_and_barrier
    tile.TileContext._drain_split_patched = True


_install_drain_split_patch()


# query blocks: 4 tiles each, 2 PSUM banks (3+1 slots of 129 cols)
BLOCKS = [(0, 4), (4, 8), (8, 12), (12, 16)]


def _build_program(loop_reps=None):
    nc = bass.Bass()

    # ---- DRAM I/O ----
    xt_d = nc.declare_dram_parameter("xt", [D, S], BF16, isOutput=False)
    # q/k/v weights arrive in device lhsT layout: [128, NT*128] where
    # col 128*d + c is W[c, 128*d + p] for d-block d (host pre-transposed,
    # so the DMA rows are contiguous 4KB)
    wq0_d = nc.declare_dram_parameter("wq0", [128, NT * 128], BF16, isOutput=False)
    wq1_d = nc.declare_dram_parameter("wq1", [128, NT * 128], BF16, isOutput=False)
    wkt_d = nc.declare_dram_parameter("wkt", [128, NT * 128], BF16, isOutput=False)
    wvt_d = nc.declare_dram_parameter("wvt", [128, NT * 128], BF16, isOutput=False)
    pt_d = nc.declare_dram_parameter("pt", [M_PER_CORE, D], BF16, isOutput=False)
    cost_d = nc.declare_dram_parameter("cost", [HD, S], BF16, isOutput=False)
    sint_d = nc.declare_dram_parameter("sint", [HD, S], BF16, isOutput=False)
    mask_d = nc.declare_dram_parameter("maskd", [128, 128], BF16, isOutput=False)
    ident_d = nc.declare_dram_parameter("ident", [128, 128], BF16, isOutput=False)
    qkc_d = nc.declare_dram_parameter("qkc", [128, 6], F32, isOutput=False)
    out_d = nc.declare_dram_parameter("partial", [S, D], BF16, isOutput=True)

    with tile.TileContext(nc) as tc:
        with ExitStack() as ctx:
            pers = ctx.enter_context(tc.tile_pool(name="pers", bufs=1))
            tp_raw = ctx.enter_context(tc.tile_pool(name="tp_raw", bufs=2))
            tp_ctmp = ctx.enter_context(tc.tile_pool(name="tp_ctmp", bufs=8))
            tp_row = ctx.enter_context(tc.tile_pool(name="tp_row", bufs=4))
            tp_probs = ctx.enter_context(tc.tile_pool(name="tp_probs",
                                                      bufs=16))
            tp_y = ctx.enter_context(tc.tile_pool(name="tp_y", bufs=3))
            tp_rec = ctx.enter_context(tc.tile_pool(name="tp_rec", bufs=3))
            tp_stage = ctx.enter_context(tc.tile_pool(name="tp_stage", bufs=3))
            ps = ctx.enter_context(
                tc.tile_pool(name="ps", bufs=1, space="PSUM"))
            dram = ctx.enter_context(
                tc.tile_pool(name="dram", bufs=1, space="DRAM"))

            def ps_tile(tag, shape=(128, 512), dtype=F32):
                b = {"big": 4, "sc": 2, "m": 2}[tag]
                return ps.tile(list(shape), dtype, tag=tag, name=tag, bufs=b)

            # ---- Phase A: loads, first-needed first, striped over the
            # three HWDGE rings; everything later-needed queues behind the
            # x tiles on the same rings.
            # strict per-ring priority order (SWDGE queues run in parallel
            # and would steal bandwidth from the critical path, so only the
            # two HWDGE rings carry loads).  x tile 0 rides alone on the SP
            # ring while the ACT ring fetches the pass-1 weights, so the
            # first matmul fires as early as possible; later-needed tensors
            # slot between the x tiles at the point they're first consumed.
            xt = [pers.tile([128, S], BF16, tag=f"xt{t}", name=f"xt{t}")
                  for t in range(NT)]

            def ld_x(eng, t):
                eng.dma_start(xt[t][:], xt_d[128 * t:128 * (t + 1), :])

            # x tiles must outpace the PE's 1.7us/d-block consumption or the
            # p-state ramp resets on every wait, so pass-1 inputs load FIRST
            # (x striped over both rings, early weight d-block pieces
            # interleaved) and everything later-needed queues after xt15.
            wq0 = pers.tile([128, NT * 128], BF16, tag="wq0", name="wq0")
            wkt_all = pers.tile([128, NT * HD], BF16, tag="wkt_all",
                                name="wkt_all")
            ld_x(nc.sync, 0)
            nc.scalar.dma_start(wq0[:, 0:256], wq0_d[:, 0:256])
            nc.scalar.dma_start(wkt_all[:, 0:256], wkt_d[:, 0:256])
            ld_x(nc.sync, 1)
            nc.scalar.dma_start(wq0[:, 256:1024], wq0_d[:, 256:1024])
            ld_x(nc.sync, 2)
            nc.scalar.dma_start(wkt_all[:, 256:1024], wkt_d[:, 256:1024])
            ld_x(nc.scalar, 3)
            nc.sync.dma_start(wq0[:, 1024:2048], wq0_d[:, 1024:2048])
            ld_x(nc.scalar, 4)
            nc.sync.dma_start(wkt_all[:, 1024:2048], wkt_d[:, 1024:2048])
            for t in range(5, NT):
                ld_x(nc.sync if t % 2 == 0 else nc.scalar, t)
            # later-needed, in first-use order
            cost = pers.tile([HD, S], BF16, tag="cost", name="cost")
            nc.scalar.dma_start(cost[:], cost_d[:])
            wq1 = pers.tile([128, NT * 128], BF16, tag="wq1", name="wq1")
            nc.sync.dma_start(wq1[:], wq1_d[:])
            sint = pers.tile([HD, S], BF16, tag="sint", name="sint")
            nc.scalar.dma_start(sint[:], sint_d[:])
            qkc = pers.tile([128, 6], F32, tag="qkc", name="qkc")
            nc.sync.dma_start(qkc[:], qkc_d[:])
            maskd = pers.tile([128, 128], BF16, tag="maskd", name="maskd")
            nc.sync.dma_start(maskd[:], mask_d[:])
            wvt_all = pers.tile([128, NT * HD], BF16, tag="wvt_all",
                                name="wvt_all")
            nc.scalar.dma_start(wvt_all[:], wvt_d[:])
            ident = pers.tile([128, 128], BF16, tag="ident", name="ident")
            nc.sync.dma_start(ident[:], ident_d[:])
            pt = []
            for h in range(QH_PER_CORE):
                w = pers.tile([128, D], BF16, tag=f"pt{h}", name=f"pt{h}")
                nc.sync.dma_start(w[:], pt_d[128 * h:128 * (h + 1), :])
                pt.append(w)

            ones_col = pers.tile([128, 1], BF16, tag="ones_col",
                                 name="ones_col")
            nc.vector.memset(ones_col[:], 1.0)
            ones_row = pers.tile([1, 128], BF16, tag="ones_row",
                                 name="ones_row")
            nc.vector.memset(ones_row[:], 1.0)
            vplus = pers.tile([128, NT * 129], BF16, tag="vplus",
                              name="vplus")
            for t in range(NT):
                nc.vector.memset(vplus[:, 129 * t + 128:129 * t + 129], 1.0)

            # persistent phase-B outputs
            qTn = [pers.tile([128, S], BF16, tag=f"qTn{h}", name=f"qTn{h}")
                   for h in range(QH_PER_CORE)]
            kTr = pers.tile([128, S], BF16, tag="kTr", name="kTr")
            invk_cols = pers.tile([128, NT], F32, tag="invk_cols",
                                  name="invk_cols")
            yT = [pers.tile([128, S], BF16, tag=f"yT{h}", name=f"yT{h}")
                  for h in range(QH_PER_CORE)]

            def copy_eng(eng, out, in_):
                if eng is nc.scalar:
                    eng.copy(out, in_)
                else:
                    eng.tensor_copy(out, in_)

            def _phases_bc():
                fillers = []     # (min_d, thunk) popped at d-boundaries

                def add_filler(thunk, min_d=2):
                    fillers.append((min_d, thunk))

                def pass_mms(units, per_d=2):
                    for d in range(NT):
                        for pss, w_of in units:
                            for c in range(4):
                                nc.tensor.matmul(
                                    pss[c][:], w_of(d),
                                    xt[d][:, 512 * c:512 * (c + 1)],
                                    start=(d == 0), stop=(d == NT - 1))
                        n = per_d
                        while (n > 0 and fillers
                               and fillers[0][0] <= d):
                            fillers.pop(0)[1]()
                            n -= 1
                    while fillers:
                        fillers.pop(0)[1]()

                def extract(pss, name, engs):
                    raw = tp_raw.tile([128, S], BF16, tag="raw", name=name)
                    for c in range(4):
                        copy_eng(engs[c % len(engs)],
                                 raw[:, 512 * c:512 * (c + 1)], pss[c][:])
                    return raw

                def chain(raw, kind, h=None, rs_tag="m", bc_tag="m",
                          rs_d0=2, bc_d0=6):
                    """Chunk-wise rope + rms-norm for a q or k unit.  The
                    rope products issue inline; the column-sum -> sqrt ->
                    reciprocal (-> broadcast -> qTn multiply) chains are
                    deferred whole into `fillers` so every producer is
                    issued before its consumers (Tile tracks dependencies
                    in issue order)."""
                    sidx = 4 if kind == "k" else 2 * h
                    rowdr = (dram.tile([1, S], F32, tag="rowdr",
                                       name="invk_row", bufs=1)
                             if kind == "k" else None)
                    sqs, qTrs, rrecs = [], [], {}
                    for c in range(4):
                        cs = slice(512 * c, 512 * (c + 1))
                        sq = tp_ctmp.tile([128, 512], BF16, tag="ctmp",
                                          name="sq")
                        nc.gpsimd.tensor_mul(sq[:], raw[:, cs], raw[:, cs])
                        sqs.append(sq)
                        sw = tp_ctmp.tile([128, 512], BF16, tag="ctmp",
                                          name="sw")
                        nc.sync.dma_start(sw[0:64, :], raw[64:128, cs])
                        nc.sync.dma_start(sw[64:128, :], raw[0:64, cs])
                        t1 = tp_ctmp.tile([128, 512], BF16, tag="ctmp",
                                          name="t1")
                        nc.vector.tensor_mul(t1[:], raw[:, cs], cost[:, cs])
                        t2 = tp_ctmp.tile([128, 512], BF16, tag="ctmp",
                                          name="t2")
                        nc.gpsimd.tensor_mul(t2[:], sw[:], sint[:, cs])
                        if kind == "k":
                            nc.vector.tensor_add(kTr[:, cs], t1[:], t2[:])
                        else:
                            qTr = tp_ctmp.tile([128, 512], BF16, tag="ctmp",
                                               name="qTr")
                            nc.vector.tensor_add(qTr[:], t1[:], t2[:])
                            qTrs.append(qTr)

                    def rs_thunk(c):
                        cs = slice(512 * c, 512 * (c + 1))
                        rp = ps_tile(rs_tag)
                        nc.tensor.matmul(rp[0:1, :], ones_col[:], sqs[c][:],
                                         start=True, stop=True)
                        rrow = tp_row.tile([1, 512], F32, tag="rrow",
                                           name="rrow")
                        nc.scalar.activation(
                            rrow[0:1, :], rp[0:1, :], AF.Sqrt,
                            scale=qkc[0:1, sidx:sidx + 1],
                            bias=qkc[0:1, sidx + 1:sidx + 2])
                        if kind == "k":
                            nc.vector.reciprocal(rrow[:], rrow[:])
                            nc.scalar.dma_start(rowdr[0:1, cs],
                                                rrow[0:1, :])
                            if c == 3:
                                nc.sync.dma_start(
                                    invk_cols[:],
                                    rowdr[0:1, :].rearrange(
                                        "one (j p) -> p (one j)", p=128))
                        else:
                            rrec = tp_row.tile([1, 512], BF16, tag="rrec",
                                               name="rrec")
                            with nc.allow_low_precision(
                                    reason="1/rms row in bf16 feeds a bf16 "
                                           "multiply; 0.4% is inside budget"):
                                nc.vector.reciprocal(rrec[:], rrow[:])
                            rrecs[c] = rrec

                    def bc_thunk(c):
                        cs = slice(512 * c, 512 * (c + 1))
                        bc = ps_tile(bc_tag)
                        nc.tensor.matmul(bc[:], ones_row[:],
                                         rrecs[c][0:1, :],
                                         start=True, stop=True)
                        nc.vector.tensor_mul(qTn[h][:, cs], qTrs[c][:],
                                             bc[:])

                    for c in range(4):
                        add_filler(lambda c=c: rs_thunk(c), rs_d0 + c)
                        if kind == "q":
                            add_filler(lambda c=c: bc_thunk(c), bc_d0 + c)

                parked = {}   # (h, blk, j) -> (probs, w, qlo)

                def score_mm(h, blk, j, tag="sc"):
                    lo, hi = BLOCKS[blk]
                    qlo = max(lo, j) * 128
                    w = hi * 128 - qlo
                    sc = ps_tile(tag)
                    nc.tensor.matmul(
                        sc[:, :w], kTr[:, 128 * j:128 * (j + 1)],
                        qTn[h][:, qlo:qlo + w], start=True, stop=True)
                    return sc, w, qlo

                def exp_of(h, blk, j, sc, w):
                    lo, hi = BLOCKS[blk]
                    pc = tp_probs.tile([128, 512], BF16, tag="probs",
                                       name="probs")
                    nc.scalar.activation(pc[:, :w], sc[:, :w], AF.Exp,
                                         scale=invk_cols[:, j:j + 1])
                    if j >= lo:
                        nc.gpsimd.tensor_mul(pc[:, 0:128], pc[:, 0:128],
                                             maskd[:])
                    return pc

                def park(h, blk, j, tag="sc"):
                    def thunk():
                        sc, w, qlo = score_mm(h, blk, j, tag)
                        parked[(h, blk, j)] = (exp_of(h, blk, j, sc, w),
                                               w, qlo)
                    return thunk

                # ---- pass 1: q-head0 (big) + k (sc+m) ----
                ps_q0 = [ps_tile("big") for _ in range(4)]
                ps_k = [ps_tile("sc"), ps_tile("sc"), ps_tile("m"),
                        ps_tile("m")]
                pass_mms(
                    [(ps_q0, lambda d: wq0[:, 128 * d:128 * (d + 1)]),
                     (ps_k, lambda d: wkt_all[:, 128 * d:128 * (d + 1)])],
                    per_d=0)
                q0raw = extract(ps_q0, "q0raw", [nc.vector, nc.scalar,
                                                 nc.vector, nc.scalar])
                kraw = extract(ps_k, "kraw", [nc.scalar, nc.vector,
                                              nc.scalar, nc.vector])
                chain(kraw, "k", rs_d0=2, bc_d0=6)
                chain(q0raw, "q", 0, rs_d0=4, bc_d0=7)
                # park head-0 scores+exps for query blocks 0/1 behind the
                # chain fillers (they run at 2a/2b d-boundaries)
                idx = 0
                for blk in (0, 1):
                    lo, hi = BLOCKS[blk]
                    for j in range(hi):
                        add_filler(park(0, blk, j), 9 + idx // 2)
                        idx += 1

                # ---- sub-pass 2a: q-head1 (big) ----
                ps_q1 = [ps_tile("big") for _ in range(4)]
                pass_mms([(ps_q1, lambda d: wq1[:, 128 * d:128 * (d + 1)])],
                         per_d=2)
                q1raw = extract(ps_q1, "q1raw", [nc.vector, nc.scalar,
                                                 nc.vector, nc.scalar])
                # q1's chain fillers and the pair-1 j=0/1 scores hide under
                # sub-pass 2b
                chain(q1raw, "q", 1, rs_d0=2, bc_d0=5)
                idx = 0
                for blk in (0, 1):
                    for j in (0, 1):
                        add_filler(park(1, blk, j), 8 + idx)
                        idx += 1

                # ---- sub-pass 2b: v (big) ----
                ps_v = [ps_tile("big") for _ in range(4)]
                pass_mms([(ps_v, lambda d: wvt_all[:, 128 * d:128 * (d + 1)])],
                         per_d=2)
                vraw = extract(ps_v, "vraw", [nc.vector, nc.scalar,
                                              nc.vector, nc.scalar])
                for t in range(NT):
                    trp = ps_tile("m", (128, 128), BF16)
                    nc.tensor.transpose(
                        trp[:], vraw[:, 128 * t:128 * (t + 1)], ident[:])
                    eng = nc.vector if t % 2 == 0 else nc.scalar
                    copy_eng(eng, vplus[:, 129 * t:129 * t + 128], trp[:])

                # ---- Phase C ----
                stage_n = [0]
                pending = []          # (i, c) output-projection chunks
                stage_tiles = {}

                def push_tile(i):
                    pending.extend((i, c) for c in range(4))

                def pop_proj(n, tag="m"):
                    for _ in range(n):
                        if not pending:
                            return
                        i, c = pending.pop(0)
                        if c == 0:
                            stage_tiles[i] = tp_stage.tile(
                                [128, D], BF16, tag="stage", name="stage")
                        stg = stage_tiles[i]
                        pp = ps_tile(tag)
                        nc.tensor.matmul(pp[:],
                                         yT[0][:, 128 * i:128 * (i + 1)],
                                         pt[0][:, 512 * c:512 * (c + 1)],
                                         start=True, stop=False)
                        nc.tensor.matmul(pp[:],
                                         yT[1][:, 128 * i:128 * (i + 1)],
                                         pt[1][:, 512 * c:512 * (c + 1)],
                                         start=False, stop=True)
                        # PSUM readers are DVE/ACT only; ACT takes every
                        # fourth copy so its exp stream stays the priority
                        if tag == "sc":   # flush: ACT is free
                            eng = nc.vector if c % 2 == 0 else nc.scalar
                        else:
                            eng = nc.scalar if (i + c) % 4 == 3 else nc.vector
                        copy_eng(eng, stg[:, 512 * c:512 * (c + 1)], pp[:])
                        if c == 3:
                            nc.sync.dma_start(
                                out_d[128 * i:128 * (i + 1), :], stg[:])
                            stage_n[0] += 1
                            del stage_tiles[i]

                def extract_bank(h, blk, bank, b, proj_arm):
                    lo, hi = BLOCKS[blk]
                    for i in range(lo + 3 * b, min(lo + 3 * b + 3, hi)):
                        sub = (i - lo) % 3
                        rec = tp_rec.tile([128, 1], F32, tag="rec",
                                          name="rec")
                        nc.vector.reciprocal(
                            rec[:], bank[:, 129 * sub + 128:129 * sub + 129])
                        y = tp_y.tile([128, 128], BF16, tag="y", name="y")
                        nc.vector.tensor_scalar_mul(
                            y[:], bank[:, 129 * sub:129 * sub + 128], rec[:])
                        trp = ps_tile("m", (128, 128), BF16)
                        nc.tensor.transpose(trp[:], y[:], ident[:])
                        nc.vector.tensor_copy(
                            yT[h][:, 128 * i:128 * (i + 1)], trp[:])
                        if proj_arm:
                            push_tile(i)

                def pv_step(h, blk, j, banks, imax, pc, wc, qloc, proj_arm):
                    lo, hi = BLOCKS[blk]
                    for i in range(max(lo, j), hi):
                        b, sub = divmod(i - lo, 3)
                        off = 128 * i - qloc
                        nc.tensor.matmul(
                            banks[b][:, 129 * sub:129 * sub + 129],
                            pc[:, off:off + 128],
                            vplus[:, 129 * j:129 * j + 129],
                            start=(j == 0 and sub == 0),
                            stop=(j == i and i == imax[b]))
                    for b in range(2):
                        if j == imax[b] and j >= lo:
                            extract_bank(h, blk, banks[b], b, proj_arm)

                # pair 0: h0 blocks 0/1 against parked probs (pure PE),
                # popping one q1-chain filler per step
                banks0 = {blk: [ps_tile("big") for _ in range(2)]
                          for blk in (0, 1)}
                sc_live = {}
                for j in range(8):
                    for blk in (0, 1):
                        lo, hi = BLOCKS[blk]
                        if j >= hi:
                            continue
                        pc, wc, qloc = parked.pop((0, blk, j))
                        imax = [lo + 2, lo + 3]
                        pv_step(0, blk, j, banks0[blk], imax, pc, wc, qloc,
                                False)
                while fillers:
                    fillers.pop(0)[1]()

                def run_pair(streams, proj_per_step):
                    banks = {}
                    imax = {}
                    for (h, blk) in streams:
                        lo, hi = BLOCKS[blk]
                        banks[(h, blk)] = [ps_tile("big") for _ in range(2)]
                        imax[(h, blk)] = [lo + 2, lo + 3]
                    for (h, blk) in streams:
                        if (h, blk, 0) in parked:
                            sc_live[(h, blk)] = parked.pop((h, blk, 0))
                        elif (h, blk) not in sc_live:
                            sc, w, qlo = score_mm(h, blk, 0)
                            sc_live[(h, blk)] = (exp_of(h, blk, 0, sc, w),
                                                 w, qlo)
                    maxhi = max(BLOCKS[blk][1] for _, blk in streams)
                    for j in range(maxhi):
                        for (h, blk) in streams:
                            lo, hi = BLOCKS[blk]
                            if j >= hi:
                                continue
                            ahead = (j + 1 < hi
                                     and (h, blk, j + 1) not in parked)
                            if ahead:
                                sc, w, qlo = score_mm(h, blk, j + 1)
                            pc, wc, qloc = sc_live.pop((h, blk))
                            pv_step(h, blk, j, banks[(h, blk)],
                                    imax[(h, blk)], pc, wc, qloc, h == 1)
                            if ahead:
                                pcn = exp_of(h, blk, j + 1, sc, w)
                                sc_live[(h, blk)] = (pcn, w, qlo)
                            elif j + 1 < hi:
                                sc_live[(h, blk)] = parked.pop((h, blk,
                                                                j + 1))
                            pop_proj(proj_per_step
                                     if len(pending) < 12 else
                                     proj_per_step + 1)

                run_pair([(1, 0), (1, 1)], 1)
                run_pair([(0, 2), (1, 2)], 1)
                run_pair([(0, 3), (1, 3)], 1)
                flip = [0]
                while pending:
                    pop_proj(1, "sc" if flip[0] % 2 == 0 else "m")
                    flip[0] += 1

            if loop_reps is None:
                _phases_bc()
            else:
                with tc.For_i(0, loop_reps, 1):
                    _phases_bc()
    return nc


_NC_CACHE = None


def _get_nc():
    global _NC_CACHE
    if _NC_CACHE is None:
        _NC_CACHE = _build_program()
    return _NC_CACHE


def _host_prep(x, Wq, Wk, Wv, Wproj, q_gain):
    """Build the 8 per-core input maps (numpy, host side)."""
    x2 = np.asarray(x, np.float32).reshape(S, D)
    xt = np.ascontiguousarray(x2.T).astype(NP_BF16)

    inv_freq = 1.0 / (BASE ** (np.arange(0, HD, 2, dtype=np.float32) / HD))
    t = np.arange(S, dtype=np.float32)
    freqs = np.outer(t, inv_freq)                     # [S, 64]
    cos_h = np.cos(freqs).T                           # [64, S]
    sin_h = np.sin(freqs).T
    cosT = np.ascontiguousarray(
        np.concatenate([cos_h, cos_h], axis=0)).astype(NP_BF16)
    sinT = np.ascontiguousarray(
        np.concatenate([sin_h, -sin_h], axis=0)).astype(NP_BF16)

    maskd = np.triu(np.ones((128, 128), np.float32)).astype(NP_BF16)
    ident = np.eye(128, dtype=np.float32).astype(NP_BF16)

    Wq = np.asarray(Wq, np.float32)
    Wk = np.asarray(Wk, np.float32)
    Wv = np.asarray(Wv, np.float32)
    Wproj = np.asarray(Wproj, np.float32)
    q_gain = np.asarray(q_gain, np.float32)

    def dev_layout(w_unit):
        # [128 out, D in] -> [128 p, NT*128] with col 128*d + c = w[c, 128d+p]
        a = w_unit.T.reshape(NT, 128, 128).transpose(1, 0, 2)
        return np.ascontiguousarray(a.reshape(128, NT * 128)).astype(NP_BF16)

    in_maps = []
    for core in range(NCORES):
        kv = core // 2
        wq0 = dev_layout(Wq[M_PER_CORE * core:M_PER_CORE * core + 128, :])
        wq1 = dev_layout(Wq[M_PER_CORE * core + 128:
                            M_PER_CORE * (core + 1), :])
        wkt = dev_layout(Wk[HD * kv:HD * (kv + 1), :])
        wvt = dev_layout(Wv[HD * kv:HD * (kv + 1), :])
        ptc = np.ascontiguousarray(
            Wproj[:, M_PER_CORE * core:M_PER_CORE * (core + 1)].T
        ).astype(NP_BF16)
        qkc = np.zeros((128, 6), np.float32)
        qkc[:, 4] = 1.0 / HD
        qkc[:, 5] = EPS
        for h in range(QH_PER_CORE):
            gain = float(q_gain[QH_PER_CORE * core + h])
            c = gain / math.sqrt(HD)
            if abs(c) < 1e-8:
                c = 1e-8
            qkc[:, 2 * h] = 1.0 / (HD * c * c)
            qkc[:, 2 * h + 1] = EPS / (c * c)
        in_maps.append({
            "xt": xt,
            "wq0": wq0,
            "wq1": wq1,
            "wkt": wkt,
            "wvt": wvt,
            "pt": ptc,
            "cost": cosT,
            "sint": sinT,
            "maskd": maskd,
            "ident": ident,
            "qkc": qkc,
        })
    return in_maps


def kernel(**inputs):
    x = inputs["x"]
    in_maps = _host_prep(x, inputs["Wq"], inputs["Wk"], inputs["Wv"],
                         inputs["Wproj"], inputs["q_gain"])
    nc = _get_nc()
    res = run_bass_kernel_spmd(nc, in_maps, list(range(NCORES)))
    out = np.zeros((S, D), np.float32)
    for i in range(NCORES):
        out += np.asarray(res.results[i]["partial"]).astype(np.float32)
    return out.reshape(1, S, D)


# revision 82
# speedup vs baseline: 1.7083x; 1.0393x over previous
"""Trainium2 Bass kernel: causal GQA self-attention (B=1, S=2048, D=2048,
H=16 q-heads, 4 kv-heads, head_dim=128) with q/k RMS-norm, full-head RoPE,
per-head q gain, and output projection.

Sharding: tensor-parallel over 8 NeuronCores. Core i owns q-heads {2i, 2i+1}
and kv-head i//2: it computes its two heads' attention output and a partial
output projection (columns 256i..256i+256 of the y-concat contracted against
Wproj), returning a full-shape [2048, 2048] bf16 partial that the host sums
across cores (the "all-reduce").

Schedule, built around keeping the PE (the bottleneck engine) gap-free:
  A: loads striped over the three HWDGE rings, first-needed first (q0/k
     weights + x tile 0 land ~3.5us in); phase-C-only tensors queue behind
     the x tiles on the same rings.
  B: projection pass 1 (q-head0 + k over all 8 PSUM banks) paced by the x
     loader, then sub-pass 2a (v) and 2b (q-head1) on 4 banks each. The
     norm/rope chains run chunk-wise behind the PE; their column-sum and
     row-broadcast matmuls (a rank-1 ones @ row product, replacing a DRAM
     round-trip) fill d-block boundaries of 2a/2b, as do head-0's scores +
     exps for query blocks 0/1 (their probs park in SBUF) and the v
     transposes.
  C: attention in four query blocks of 4 tiles per head. Pair 0 (h0 blocks
     0/1) is pure PV against the parked probs; pairs (h1 b0/b1), (h0+h1 b2),
     (h0+h1 b3) run j-interleaved with scores issued one step ahead of PV so
     each stream's exp hides under the other's PE work. Output-projection
     chunks spread one per stream step from a pending queue (tiles enqueue
     as both heads' y land), staging to bf16 and DMAing a full row-tile out.
"""

import math
from contextlib import ExitStack

import numpy as np
import ml_dtypes

import concourse.bass as bass
import concourse.tile as tile
from concourse import mybir, library_config
from concourse.bass_isa import ReduceOp
from concourse.bass_utils import run_bass_kernel_spmd

BF16 = mybir.dt.bfloat16
F32 = mybir.dt.float32
NP_BF16 = ml_dtypes.bfloat16

S = 2048
D = 2048
H = 16
HKV = 4
HD = 128
NCORES = 8
QH_PER_CORE = H // NCORES          # 2
M_PER_CORE = QH_PER_CORE * HD      # 256
NT = S // 128                      # 16 s-tiles / d-blocks
BASE = 10000.0
EPS = float(np.finfo(np.float32).eps)

AF = mybir.ActivationFunctionType
ALU = mybir.AluOpType


_MAXW = 1  # max sync-wait commands this walrus accepts per instruction


def _install_drain_split_patch():
    """The walrus build here only accepts <=2 sync-wait commands per
    instruction.  Tile attaches one wait per producer semaphore, which can be
    many.  Split the excess onto same-engine NoOps committed immediately
    before the instruction (same program point, so ordering semantics are
    unchanged), and likewise chain the kernel-tail drain."""
    from concourse.vector_clock import ScopedClock
    import bass_rust

    if getattr(tile.TileContext, "_drain_split_patched", False):
        return

    orig_commit = tile.TileContext._commit_instruction

    def _commit_instruction(self, inst, lazy_reg_writes=True):
        si = getattr(inst, "sync_info", None)
        if (si is not None and si.on_wait and len(si.on_wait) > _MAXW
                and inst.engine != mybir.EngineType.Unassigned
                and isinstance(inst, mybir.Instruction)):
            waits = list(si.on_wait)
            excess, keep = waits[:-_MAXW], waits[-_MAXW:]
            for k in range(0, len(excess), _MAXW):
                nop = mybir.InstNoOp(
                    name=f"{inst.name}-wsplit{k}", ins=[], outs=[])
                nop.engine = inst.engine
                nop.sync_info = bass_rust.SyncInfo(
                    on_wait=excess[k:k + _MAXW], on_update=[])
                self._add_instruction(nop)
            si.on_wait = keep
            inst.sync_info = si
        return orig_commit(self, inst, lazy_reg_writes)

    tile.TileContext._commit_instruction = _commit_instruction

    def _drain_and_barrier(self, tick_clock, wait_clock):
        nc = self.nc
        drain_inst = nc.sync.drain()
        wait_clock.add_sem_waits(
            drain_inst.ins, ScopedClock({None: tick_clock.global_clock}))
        mi = drain_inst.ins
        si = mi.sync_info
        if si is not None and si.on_wait and len(si.on_wait) > 1:
            waits = list(si.on_wait)
            si.on_wait = waits[:1]
            mi.sync_info = si
            for w in waits[1:]:
                d2 = nc.sync.drain()
                s2 = d2.ins.sync_info
                if s2 is None:
                    s2 = type(si)(on_wait=[w], on_update=[])
                else:
                    s2.on_wait = [w]
                d2.ins.sync_info = s2
        nc.all_engine_barrier()
        assert self.sems is not None
        popped = nc._tile_sem_poison_stack.pop()
        assert popped is self._sem_poison
        nc.clear_and_free_semaphores(list(self.sems.allocated().values()))
        nc.all_engine_barrier()

    tile.TileContext._drain_and_barrier = _drain_and_barrier
    tile.TileContext._drain_split_patched = True


_install_drain_split_patch()


# query blocks: 4 tiles each, 2 PSUM banks (3+1 slots of 129 cols)
BLOCKS = [(0, 4), (4, 8), (8, 12), (12, 16)]


def _build_program(loop_reps=None):
    nc = bass.Bass()

    # ---- DRAM I/O ----
    xt_d = nc.declare_dram_parameter("xt", [D, S], BF16, isOutput=False)
    # q/k/v weights arrive in device lhsT layout: [128, NT*128] where
    # col 128*d + c is W[c, 128*d + p] for d-block d (host pre-transposed,
    # so the DMA rows are contiguous 4KB)
    wq0_d = nc.declare_dram_parameter("wq0", [128, NT * 128], BF16, isOutput=False)
    wq1_d = nc.declare_dram_parameter("wq1", [128, NT * 128], BF16, isOutput=False)
    wkt_d = nc.declare_dram_parameter("wkt", [128, NT * 128], BF16, isOutput=False)
    wvt_d = nc.declare_dram_parameter("wvt", [128, NT * 128], BF16, isOutput=False)
    pt_d = nc.declare_dram_parameter("pt", [M_PER_CORE, D], BF16, isOutput=False)
    cost_d = nc.declare_dram_parameter("cost", [HD, S], BF16, isOutput=False)
    sint_d = nc.declare_dram_parameter("sint", [HD, S], BF16, isOutput=False)
    mask_d = nc.declare_dram_parameter("maskd", [128, 128], BF16, isOutput=False)
    ident_d = nc.declare_dram_parameter("ident", [128, 128], BF16, isOutput=False)
    qkc_d = nc.declare_dram_parameter("qkc", [128, 6], F32, isOutput=False)
    out_d = nc.declare_dram_parameter("partial", [S, D], BF16, isOutput=True)

    with tile.TileContext(nc) as tc:
        with ExitStack() as ctx:
            pers = ctx.enter_context(tc.tile_pool(name="pers", bufs=1))
            tp_raw = ctx.enter_context(tc.tile_pool(name="tp_raw", bufs=2))
            tp_ctmp = ctx.enter_context(tc.tile_pool(name="tp_ctmp", bufs=8))
            tp_row = ctx.enter_context(tc.tile_pool(name="tp_row", bufs=4))
            tp_probs = ctx.enter_context(tc.tile_pool(name="tp_probs",
                                                      bufs=16))
            tp_y = ctx.enter_context(tc.tile_pool(name="tp_y", bufs=3))
            tp_rec = ctx.enter_context(tc.tile_pool(name="tp_rec", bufs=3))
            tp_stage = ctx.enter_context(tc.tile_pool(name="tp_stage", bufs=3))
            ps = ctx.enter_context(
                tc.tile_pool(name="ps", bufs=1, space="PSUM"))
            dram = ctx.enter_context(
                tc.tile_pool(name="dram", bufs=1, space="DRAM"))

            def ps_tile(tag, shape=(128, 512), dtype=F32):
                b = {"big": 4, "sc": 2, "m": 2}[tag]
                return ps.tile(list(shape), dtype, tag=tag, name=tag, bufs=b)

            # ---- Phase A: loads, first-needed first, striped over the
            # three HWDGE rings; everything later-needed queues behind the
            # x tiles on the same rings.
            # strict per-ring priority order (SWDGE queues run in parallel
            # and would steal bandwidth from the critical path, so only the
            # two HWDGE rings carry loads).  x tile 0 rides alone on the SP
            # ring while the ACT ring fetches the pass-1 weights, so the
            # first matmul fires as early as possible; later-needed tensors
            # slot between the x tiles at the point they're first consumed.
            xt = [pers.tile([128, S], BF16, tag=f"xt{t}", name=f"xt{t}")
                  for t in range(NT)]

            def ld_x(eng, t):
                eng.dma_start(xt[t][:], xt_d[128 * t:128 * (t + 1), :])

            # x tiles must outpace the PE's 1.7us/d-block consumption or the
            # p-state ramp resets on every wait, so pass-1 inputs load FIRST
            # (x striped over both rings, early weight d-block pieces
            # interleaved) and everything later-needed queues after xt15.
            wq0 = pers.tile([128, NT * 128], BF16, tag="wq0", name="wq0")
            wkt_all = pers.tile([128, NT * HD], BF16, tag="wkt_all",
                                name="wkt_all")
            # x tile 0 in two halves so the d-block-0 matmuls start sooner
            nc.sync.dma_start(xt[0][:, 0:1024], xt_d[0:128, 0:1024])
            nc.sync.dma_start(xt[0][:, 1024:2048], xt_d[0:128, 1024:2048])
            nc.scalar.dma_start(wq0[:, 0:256], wq0_d[:, 0:256])
            nc.scalar.dma_start(wkt_all[:, 0:256], wkt_d[:, 0:256])
            ld_x(nc.sync, 1)
            nc.scalar.dma_start(wq0[:, 256:1024], wq0_d[:, 256:1024])
            ld_x(nc.sync, 2)
            nc.scalar.dma_start(wkt_all[:, 256:1024], wkt_d[:, 256:1024])
            ld_x(nc.scalar, 3)
            nc.sync.dma_start(wq0[:, 1024:2048], wq0_d[:, 1024:2048])
            ld_x(nc.scalar, 4)
            nc.sync.dma_start(wkt_all[:, 1024:2048], wkt_d[:, 1024:2048])
            for t in range(5, NT):
                ld_x(nc.sync if t % 2 == 0 else nc.scalar, t)
            # later-needed, in first-use order
            cost = pers.tile([HD, S], BF16, tag="cost", name="cost")
            nc.scalar.dma_start(cost[:], cost_d[:])
            wq1 = pers.tile([128, NT * 128], BF16, tag="wq1", name="wq1")
            nc.sync.dma_start(wq1[:], wq1_d[:])
            sint = pers.tile([HD, S], BF16, tag="sint", name="sint")
            nc.scalar.dma_start(sint[:], sint_d[:])
            qkc = pers.tile([128, 6], F32, tag="qkc", name="qkc")
            nc.sync.dma_start(qkc[:], qkc_d[:])
            maskd = pers.tile([128, 128], BF16, tag="maskd", name="maskd")
            nc.sync.dma_start(maskd[:], mask_d[:])
            wvt_all = pers.tile([128, NT * HD], BF16, tag="wvt_all",
                                name="wvt_all")
            nc.scalar.dma_start(wvt_all[:], wvt_d[:])
            ident = pers.tile([128, 128], BF16, tag="ident", name="ident")
            nc.sync.dma_start(ident[:], ident_d[:])
            pt = []
            for h in range(QH_PER_CORE):
                w = pers.tile([128, D], BF16, tag=f"pt{h}", name=f"pt{h}")
                nc.sync.dma_start(w[:], pt_d[128 * h:128 * (h + 1), :])
                pt.append(w)

            ones_col = pers.tile([128, 1], BF16, tag="ones_col",
                                 name="ones_col")
            nc.vector.memset(ones_col[:], 1.0)
            ones_row = pers.tile([1, 128], BF16, tag="ones_row",
                                 name="ones_row")
            nc.vector.memset(ones_row[:], 1.0)
            vplus = pers.tile([128, NT * 129], BF16, tag="vplus",
                              name="vplus")
            for t in range(NT):
                nc.vector.memset(vplus[:, 129 * t + 128:129 * t + 129], 1.0)

            # persistent phase-B outputs
            qTn = [pers.tile([128, S], BF16, tag=f"qTn{h}", name=f"qTn{h}")
                   for h in range(QH_PER_CORE)]
            kTr = pers.tile([128, S], BF16, tag="kTr", name="kTr")
            invk_cols = pers.tile([128, NT], F32, tag="invk_cols",
                                  name="invk_cols")
            yT = [pers.tile([128, S], BF16, tag=f"yT{h}", name=f"yT{h}")
                  for h in range(QH_PER_CORE)]

            def copy_eng(eng, out, in_):
                if eng is nc.scalar:
                    eng.copy(out, in_)
                else:
                    eng.tensor_copy(out, in_)

            def _phases_bc():
                fillers = []     # (min_d, thunk) popped at d-boundaries

                def add_filler(thunk, min_d=2):
                    fillers.append((min_d, thunk))

                def pass_mms(units, per_d=2):
                    for d in range(NT):
                        for pss, w_of in units:
                            for c in range(4):
                                nc.tensor.matmul(
                                    pss[c][:], w_of(d),
                                    xt[d][:, 512 * c:512 * (c + 1)],
                                    start=(d == 0), stop=(d == NT - 1))
                        n = per_d
                        while (n > 0 and fillers
                               and fillers[0][0] <= d):
                            fillers.pop(0)[1]()
                            n -= 1
                    while fillers:
                        fillers.pop(0)[1]()

                def extract(pss, name, engs):
                    raw = tp_raw.tile([128, S], BF16, tag="raw", name=name)
                    for c in range(4):
                        copy_eng(engs[c % len(engs)],
                                 raw[:, 512 * c:512 * (c + 1)], pss[c][:])
                    return raw

                def chain(raw, kind, h=None, rs_tag="m", bc_tag="m",
                          rs_d0=2, bc_d0=6, park_list=None):
                    """Chunk-wise rope + rms-norm for a q or k unit.  The
                    rope products issue inline; the column-sum -> sqrt ->
                    reciprocal (-> broadcast -> qTn multiply) chains are
                    deferred whole into `fillers` so every producer is
                    issued before its consumers (Tile tracks dependencies
                    in issue order)."""
                    sidx = 4 if kind == "k" else 2 * h
                    rowdr = (dram.tile([1, S], F32, tag="rowdr",
                                       name="invk_row", bufs=1)
                             if kind == "k" else None)
                    # squares first (DVE, cheap) — they gate the rowsum
                    # fillers; ropes follow
                    sqs, qTrs, rrecs = [], [], {}
                    for c in range(4):
                        cs = slice(512 * c, 512 * (c + 1))
                        sq = tp_ctmp.tile([128, 512], BF16, tag="sq",
                                          bufs=4, name="sq")
                        nc.vector.tensor_mul(sq[:], raw[:, cs], raw[:, cs])
                        sqs.append(sq)
                    for c in range(4):
                        cs = slice(512 * c, 512 * (c + 1))
                        sw = tp_ctmp.tile([128, 512], BF16, tag="ctmp",
                                          name="sw")
                        nc.sync.dma_start(sw[0:64, :], raw[64:128, cs])
                        nc.sync.dma_start(sw[64:128, :], raw[0:64, cs])
                        t1 = tp_ctmp.tile([128, 512], BF16, tag="ctmp",
                                          name="t1")
                        nc.vector.tensor_mul(t1[:], raw[:, cs], cost[:, cs])
                        t2 = tp_ctmp.tile([128, 512], BF16, tag="ctmp",
                                          name="t2")
                        nc.gpsimd.tensor_mul(t2[:], sw[:], sint[:, cs])
                        if kind == "k":
                            nc.vector.tensor_add(kTr[:, cs], t1[:], t2[:])
                        else:
                            qTr = tp_ctmp.tile([128, 512], BF16, tag="qtr",
                                               bufs=4, name="qTr")
                            nc.vector.tensor_add(qTr[:], t1[:], t2[:])
                            qTrs.append(qTr)

                    def rs_thunk(c):
                        cs = slice(512 * c, 512 * (c + 1))
                        rp = ps_tile(rs_tag)
                        nc.tensor.matmul(rp[0:1, :], ones_col[:], sqs[c][:],
                                         start=True, stop=True)
                        rrow = tp_row.tile([1, 512], F32, tag="rrow",
                                           name="rrow")
                        nc.scalar.activation(
                            rrow[0:1, :], rp[0:1, :], AF.Sqrt,
                            scale=qkc[0:1, sidx:sidx + 1],
                            bias=qkc[0:1, sidx + 1:sidx + 2])
                        if kind == "k":
                            nc.vector.reciprocal(rrow[:], rrow[:])
                            nc.scalar.dma_start(rowdr[0:1, cs],
                                                rrow[0:1, :])
                            if c == 3:
                                nc.sync.dma_start(
                                    invk_cols[:],
                                    rowdr[0:1, :].rearrange(
                                        "one (j p) -> p (one j)", p=128))
                        else:
                            rrec = tp_row.tile([1, 512], BF16, tag="rrec",
                                               name="rrec")
                            with nc.allow_low_precision(
                                    reason="1/rms row in bf16 feeds a bf16 "
                                           "multiply; 0.4% is inside budget"):
                                nc.vector.reciprocal(rrec[:], rrow[:])
                            rrecs[c] = rrec

                    def bc_thunk(c):
                        cs = slice(512 * c, 512 * (c + 1))
                        bc = ps_tile(bc_tag)
                        nc.tensor.matmul(bc[:], ones_row[:],
                                         rrecs[c][0:1, :],
                                         start=True, stop=True)
                        nc.vector.tensor_mul(qTn[h][:, cs], qTrs[c][:],
                                             bc[:])

                    for c in range(4):
                        add_filler(lambda c=c: rs_thunk(c), rs_d0 + c // 2)
                        if kind == "q":
                            add_filler(lambda c=c: bc_thunk(c),
                                       bc_d0 + c // 2)

                parked = {}   # (h, blk, j) -> (probs, w, qlo)

                def score_mm(h, blk, j, tag="sc"):
                    lo, hi = BLOCKS[blk]
                    qlo = max(lo, j) * 128
                    w = hi * 128 - qlo
                    sc = ps_tile(tag)
                    nc.tensor.matmul(
                        sc[:, :w], kTr[:, 128 * j:128 * (j + 1)],
                        qTn[h][:, qlo:qlo + w], start=True, stop=True)
                    return sc, w, qlo

                def exp_of(h, blk, j, sc, w):
                    lo, hi = BLOCKS[blk]
                    pc = tp_probs.tile([128, 512], BF16, tag="probs",
                                       name="probs")
                    nc.scalar.activation(pc[:, :w], sc[:, :w], AF.Exp,
                                         scale=invk_cols[:, j:j + 1])
                    if j >= lo:
                        nc.gpsimd.tensor_mul(pc[:, 0:128], pc[:, 0:128],
                                             maskd[:])
                    return pc

                def park(h, blk, j, tag="sc"):
                    def thunk():
                        sc, w, qlo = score_mm(h, blk, j, tag)
                        parked[(h, blk, j)] = (exp_of(h, blk, j, sc, w),
                                               w, qlo)
                    return thunk

                # ---- pass 1: q-head0 (big) + k (sc+m) ----
                ps_q0 = [ps_tile("big") for _ in range(4)]
                ps_k = [ps_tile("sc"), ps_tile("sc"), ps_tile("m"),
                        ps_tile("m")]
                pass_mms(
                    [(ps_q0, lambda d: wq0[:, 128 * d:128 * (d + 1)]),
                     (ps_k, lambda d: wkt_all[:, 128 * d:128 * (d + 1)])],
                    per_d=0)
                q0raw = extract(ps_q0, "q0raw", [nc.vector, nc.scalar,
                                                 nc.vector, nc.scalar])
                kraw = extract(ps_k, "kraw", [nc.scalar, nc.vector,
                                              nc.scalar, nc.vector])
                chain(kraw, "k", rs_d0=2)
                chain(q0raw, "q", 0, rs_d0=5, bc_d0=7)
                # park head-0 scores+exps for query blocks 0/1 behind the
                # chain fillers (they run at 2a/2b d-boundaries)
                idx = 0
                for blk in (0, 1):
                    lo, hi = BLOCKS[blk]
                    for j in range(hi):
                        add_filler(park(0, blk, j), 9 + idx // 2)
                        idx += 1

                # ---- sub-pass 2a: q-head1 (big) ----
                ps_q1 = [ps_tile("big") for _ in range(4)]
                pass_mms([(ps_q1, lambda d: wq1[:, 128 * d:128 * (d + 1)])],
                         per_d=2)
                q1raw = extract(ps_q1, "q1raw", [nc.vector, nc.scalar,
                                                 nc.vector, nc.scalar])
                # q1's chain fillers and the pair-1 j=0/1 scores hide under
                # sub-pass 2b
                chain(q1raw, "q", 1, rs_d0=2, bc_d0=5)
                idx = 0
                for blk in (0, 1):
                    for j in (0, 1):
                        add_filler(park(1, blk, j), 7 + idx // 2)
                        idx += 1

                # ---- sub-pass 2b: v (big) ----
                ps_v = [ps_tile("big") for _ in range(4)]
                pass_mms([(ps_v, lambda d: wvt_all[:, 128 * d:128 * (d + 1)])],
                         per_d=2)
                vraw = extract(ps_v, "vraw", [nc.vector, nc.scalar,
                                              nc.vector, nc.scalar])
                for t in range(NT):
                    trp = ps_tile("m", (128, 128), BF16)
                    nc.tensor.transpose(
                        trp[:], vraw[:, 128 * t:128 * (t + 1)], ident[:])
                    eng = nc.vector if t % 2 == 0 else nc.scalar
                    copy_eng(eng, vplus[:, 129 * t:129 * t + 128], trp[:])

                # ---- Phase C ----
                stage_n = [0]
                pending = []          # (i, c) output-projection chunks
                stage_tiles = {}

                def push_tile(i):
                    pending.extend((i, c) for c in range(4))

                flush_tiles = set()

                def pop_proj(n, tag="m", flush=False):
                    for _ in range(n):
                        if not pending:
                            return
                        i, c = pending.pop(0)
                        if c == 0:
                            stage_tiles[i] = tp_stage.tile(
                                [128, D], BF16, tag="stage", name="stage")
                            if flush:
                                flush_tiles.add(i)
                        stg = stage_tiles[i]
                        pp = ps_tile(tag)
                        nc.tensor.matmul(pp[:],
                                         yT[0][:, 128 * i:128 * (i + 1)],
                                         pt[0][:, 512 * c:512 * (c + 1)],
                                         start=True, stop=False)
                        nc.tensor.matmul(pp[:],
                                         yT[1][:, 128 * i:128 * (i + 1)],
                                         pt[1][:, 512 * c:512 * (c + 1)],
                                         start=False, stop=True)
                        # PSUM readers are DVE/ACT only; ACT takes every
                        # fourth copy in-pair so its exp stream stays the
                        # priority, and alternates in the flush (exps done)
                        if tag == "sc":
                            eng = nc.vector if c % 2 == 0 else nc.scalar
                        else:
                            eng = nc.scalar if (i + c) % 4 == 3 else nc.vector
                        copy_eng(eng, stg[:, 512 * c:512 * (c + 1)], pp[:])
                        if i in flush_tiles:
                            # per-chunk DMA so the final tiles drain as
                            # their chunks land instead of all at the end
                            nc.sync.dma_start(
                                out_d[128 * i:128 * (i + 1),
                                      512 * c:512 * (c + 1)],
                                stg[:, 512 * c:512 * (c + 1)])
                            if c == 3:
                                stage_n[0] += 1
                                del stage_tiles[i]
                        elif c == 3:
                            nc.sync.dma_start(
                                out_d[128 * i:128 * (i + 1), :], stg[:])
                            stage_n[0] += 1
                            del stage_tiles[i]

                def extract_bank(h, blk, bank, b, proj_arm):
                    lo, hi = BLOCKS[blk]
                    for i in range(lo + 3 * b, min(lo + 3 * b + 3, hi)):
                        sub = (i - lo) % 3
                        rec = tp_rec.tile([128, 1], F32, tag="rec",
                                          name="rec")
                        nc.vector.reciprocal(
                            rec[:], bank[:, 129 * sub + 128:129 * sub + 129])
                        y = tp_y.tile([128, 128], BF16, tag="y", name="y")
                        nc.vector.tensor_scalar_mul(
                            y[:], bank[:, 129 * sub:129 * sub + 128], rec[:])
                        trp = ps_tile("m", (128, 128), BF16)
                        nc.tensor.transpose(trp[:], y[:], ident[:])
                        nc.vector.tensor_copy(
                            yT[h][:, 128 * i:128 * (i + 1)], trp[:])
                        if proj_arm:
                            push_tile(i)

                def pv_step(h, blk, j, banks, imax, pc, wc, qloc, proj_arm):
                    lo, hi = BLOCKS[blk]
                    for i in range(max(lo, j), hi):
                        b, sub = divmod(i - lo, 3)
                        off = 128 * i - qloc
                        nc.tensor.matmul(
                            banks[b][:, 129 * sub:129 * sub + 129],
                            pc[:, off:off + 128],
                            vplus[:, 129 * j:129 * j + 129],
                            start=(j == 0 and sub == 0),
                            stop=(j == i and i == imax[b]))
                    for b in range(2):
                        if j == imax[b] and j >= lo:
                            extract_bank(h, blk, banks[b], b, proj_arm)

                # pair 0: h0 blocks 0/1 against parked probs (pure PE),
                # popping one q1-chain filler per step
                banks0 = {blk: [ps_tile("big") for _ in range(2)]
                          for blk in (0, 1)}
                sc_live = {}
                for j in range(8):
                    for blk in (0, 1):
                        lo, hi = BLOCKS[blk]
                        if j >= hi:
                            continue
                        pc, wc, qloc = parked.pop((0, blk, j))
                        imax = [lo + 2, lo + 3]
                        pv_step(0, blk, j, banks0[blk], imax, pc, wc, qloc,
                                False)
                while fillers:
                    fillers.pop(0)[1]()

                def run_pair(streams, proj_per_step):
                    banks = {}
                    imax = {}
                    for (h, blk) in streams:
                        lo, hi = BLOCKS[blk]
                        banks[(h, blk)] = [ps_tile("big") for _ in range(2)]
                        imax[(h, blk)] = [lo + 2, lo + 3]
                    for (h, blk) in streams:
                        if (h, blk, 0) in parked:
                            sc_live[(h, blk)] = parked.pop((h, blk, 0))
                        elif (h, blk) not in sc_live:
                            sc, w, qlo = score_mm(h, blk, 0)
                            sc_live[(h, blk)] = (exp_of(h, blk, 0, sc, w),
                                                 w, qlo)
                    maxhi = max(BLOCKS[blk][1] for _, blk in streams)
                    for j in range(maxhi):
                        for (h, blk) in streams:
                            lo, hi = BLOCKS[blk]
                            if j >= hi:
                                continue
                            ahead = (j + 1 < hi
                                     and (h, blk, j + 1) not in parked)
                            if ahead:
                                sc, w, qlo = score_mm(h, blk, j + 1)
                            pc, wc, qloc = sc_live.pop((h, blk))
                            pv_step(h, blk, j, banks[(h, blk)],
                                    imax[(h, blk)], pc, wc, qloc, h == 1)
                            if ahead:
                                pcn = exp_of(h, blk, j + 1, sc, w)
                                sc_live[(h, blk)] = (pcn, w, qlo)
                            elif j + 1 < hi:
                                sc_live[(h, blk)] = parked.pop((h, blk,
                                                                j + 1))
                            pop_proj(proj_per_step
                                     if len(pending) < 12 else
                                     proj_per_step + 1)

                run_pair([(1, 0), (1, 1)], 1)
                run_pair([(0, 2), (1, 2)], 1)
                run_pair([(0, 3), (1, 3)], 1)
                flip = [0]
                while pending:
                    pop_proj(1, "sc" if flip[0] % 2 == 0 else "m",
                             flush=True)
                    flip[0] += 1

            if loop_reps is None:
                _phases_bc()
            else:
                with tc.For_i(0, loop_reps, 1):
                    _phases_bc()
    return nc


_NC_CACHE = None


def _get_nc():
    global _NC_CACHE
    if _NC_CACHE is None:
        _NC_CACHE = _build_program()
    return _NC_CACHE


def _host_prep(x, Wq, Wk, Wv, Wproj, q_gain):
    """Build the 8 per-core input maps (numpy, host side)."""
    x2 = np.asarray(x, np.float32).reshape(S, D)
    xt = np.ascontiguousarray(x2.T).astype(NP_BF16)

    inv_freq = 1.0 / (BASE ** (np.arange(0, HD, 2, dtype=np.float32) / HD))
    t = np.arange(S, dtype=np.float32)
    freqs = np.outer(t, inv_freq)                     # [S, 64]
    cos_h = np.cos(freqs).T                           # [64, S]
    sin_h = np.sin(freqs).T
    cosT = np.ascontiguousarray(
        np.concatenate([cos_h, cos_h], axis=0)).astype(NP_BF16)
    sinT = np.ascontiguousarray(
        np.concatenate([sin_h, -sin_h], axis=0)).astype(NP_BF16)

    maskd = np.triu(np.ones((128, 128), np.float32)).astype(NP_BF16)
    ident = np.eye(128, dtype=np.float32).astype(NP_BF16)

    Wq = np.asarray(Wq, np.float32)
    Wk = np.asarray(Wk, np.float32)
    Wv = np.asarray(Wv, np.float32)
    Wproj = np.asarray(Wproj, np.float32)
    q_gain = np.asarray(q_gain, np.float32)

    def dev_layout(w_unit):
        # [128 out, D in] -> [128 p, NT*128] with col 128*d + c = w[c, 128d+p]
        a = w_unit.T.reshape(NT, 128, 128).transpose(1, 0, 2)
        return np.ascontiguousarray(a.reshape(128, NT * 128)).astype(NP_BF16)

    in_maps = []
    for core in range(NCORES):
        kv = core // 2
        wq0 = dev_layout(Wq[M_PER_CORE * core:M_PER_CORE * core + 128, :])
        wq1 = dev_layout(Wq[M_PER_CORE * core + 128:
                            M_PER_CORE * (core + 1), :])
        wkt = dev_layout(Wk[HD * kv:HD * (kv + 1), :])
        wvt = dev_layout(Wv[HD * kv:HD * (kv + 1), :])
        ptc = np.ascontiguousarray(
            Wproj[:, M_PER_CORE * core:M_PER_CORE * (core + 1)].T
        ).astype(NP_BF16)
        qkc = np.zeros((128, 6), np.float32)
        qkc[:, 4] = 1.0 / HD
        qkc[:, 5] = EPS
        for h in range(QH_PER_CORE):
            gain = float(q_gain[QH_PER_CORE * core + h])
            c = gain / math.sqrt(HD)
            if abs(c) < 1e-8:
                c = 1e-8
            qkc[:, 2 * h] = 1.0 / (HD * c * c)
            qkc[:, 2 * h + 1] = EPS / (c * c)
        in_maps.append({
            "xt": xt,
            "wq0": wq0,
            "wq1": wq1,
            "wkt": wkt,
            "wvt": wvt,
            "pt": ptc,
            "cost": cosT,
            "sint": sinT,
            "maskd": maskd,
            "ident": ident,
            "qkc": qkc,
        })
    return in_maps


def kernel(**inputs):
    x = inputs["x"]
    in_maps = _host_prep(x, inputs["Wq"], inputs["Wk"], inputs["Wv"],
                         inputs["Wproj"], inputs["q_gain"])
    nc = _get_nc()
    res = run_bass_kernel_spmd(nc, in_maps, list(range(NCORES)))
    out = np.zeros((S, D), np.float32)
    for i in range(NCORES):
        out += np.asarray(res.results[i]["partial"]).astype(np.float32)
    return out.reshape(1, S, D)
